# revision 44
# baseline (speedup 1.0000x reference)
"""Trainium2 Bass kernel for nn_CrossAttention (B=2, S=2048, E=1024, H=16, ctx=768).

Sharding: 4-way tensor-parallel over heads x 2-way data-parallel over batch.
Core c handles batch c//4 and heads 4*(c%4) .. 4*(c%4)+3.

Software-pipelined single-pass design, v2 (fine-grained schedule):
  PE is the near-saturated engine (~160us of matmul work vs ~142us of ACT
  exp work), so the schedule aims for: fast start (memsets before DMA
  issues, host-prepped contiguous DMA layouts split across the
  scalar/gpsimd/SP issue queues, chunk-0 PSUM drains on the idle ACT),
  per-t filler units of <=2 matmuls spread by deadline (v-projection
  split per head-pair so window 0 only carries its own heads' v; drains
  land >=2 slots before their consumer; o-units >=2 slots after the
  normalize that feeds them), next-window scores prefetched before av so
  the ACT stream crosses window boundaries without a bubble, and a
  compressed tail (full-width normalize with the denominator copy on the
  idle ACT engine, out-proj casts alternating ACT/DVE into merged
  full-row staging, output DMAs on the scalar+SP queues).

Per-core dataflow (fp16 operands, fp32 PSUM):
  qT/kT = W-stationary projections producing [dh, S] layouts (bias added
          during the PSUM->SBUF drain; DVE in-window, ACT for chunk 0)
  v2    = ctxT-tile-stationary projection into [S-tile, head, v|ones],
          one unit per (t, head-pair)
  scT   = kT-tile x qT (K=64); head pair emitted on PE row groups 0/64
  exp   = ScalarE, fused 1/sqrt(dh) scale, PSUM -> SBUF fp16, one
          [128,1024] tile per (t, head-pair); 1-deep scores lookahead
  av/Z  = [v_h | ones] stationary: rows 0:64 unnormalized out.T, rows
          64:128 softmax denominator, pair packed in one [128,1024] PSUM
  norm  = DVE: recip of denominator rows, multiply into avT fp16
  out   = avT-stationary x Wo, fp16 partial [S, E] per core

Host side: pre-transpose x/context, slice weights per head group, fp16
cast; sum the 4 per-batch fp16 partials + bo in fp32 on host.
"""
import numpy as np

import concourse.bass as bass
import concourse.mybir as mybir
import concourse.tile as tile
from concourse import bacc, bass_utils

F16 = mybir.dt.float16
F32 = mybir.dt.float32
AF = mybir.ActivationFunctionType
OP = mybir.AluOpType

B, S, E, C, H, DH = 2, 2048, 1024, 768, 16, 64
N_CORES = 8
GROUPS = 4            # head groups (tensor parallel)
HPG = H // GROUPS     # heads per group = 4
DSL = HPG * DH        # feature slice per core = 256
KT_E = E // 128       # 8 k-tiles for x projections
KT_C = C // 128       # 6 k-tiles for context projections
SCK = S // 512        # 4 s-chunks
TT = S // 128         # 16 t-tiles

_NC_CACHE = {}


def _build_nc():
    nc = bacc.Bacc("TRN2", target_bir_lowering=False, debug=False,
                   num_devices=N_CORES)

    # all inputs host-permuted into their exact SBUF layouts so every DMA
    # moves multi-KB contiguous lines (strided small-line DMAs measured
    # ~210 GB/s vs ~330+ contiguous)
    xTc = nc.dram_tensor("xTc", [SCK, 128, KT_E, 512], F16,
                         kind="ExternalInput").ap()
    ctxc = nc.dram_tensor("ctxc", [SCK, 128, KT_C, 512], F16,
                          kind="ExternalInput").ap()
    wq = nc.dram_tensor("wq", [128, KT_E, DSL], F16,
                        kind="ExternalInput").ap()
    wk = nc.dram_tensor("wk", [128, KT_C, DSL], F16,
                        kind="ExternalInput").ap()
    wv = nc.dram_tensor("wv", [128, KT_C, DSL], F16,
                        kind="ExternalInput").ap()
    wo = nc.dram_tensor("wo", [128, 2, E], F16, kind="ExternalInput").ap()
    bqk = nc.dram_tensor("bqk", [128, 4], F32, kind="ExternalInput").ap()
    bvr = nc.dram_tensor("bvr", [128, DSL], F32, kind="ExternalInput").ap()
    out = nc.dram_tensor("out", [S, E], F16, kind="ExternalOutput").ap()

    with tile.TileContext(nc) as tc:
        with (
            tc.tile_pool(name="const", bufs=1) as cpool,
            tc.tile_pool(name="ex", bufs=4) as expool,
            tc.tile_pool(name="os", bufs=4) as ospool,
            tc.tile_pool(name="rz", bufs=1) as rzpool,
        ):
            wq_sb = cpool.tile([128, KT_E, DSL], F16)
            wk_sb = cpool.tile([128, KT_C, DSL], F16)
            wv_sb = cpool.tile([128, KT_C, DSL], F16)
            wo_sb = cpool.tile([128, 2, E], F16)
            bqk_sb = cpool.tile([128, 4], F32)
            bvr_sb = cpool.tile([128, DSL], F32)
            warm_sb = cpool.tile([1, 8], F32)
            ctxT_sb = cpool.tile([128, KT_C, S], F16)
            xT_sb = cpool.tile([128, KT_E, S], F16)

            qT_sb = cpool.tile([128, 2, S], F16)
            kT_sb = cpool.tile([128, 2, S], F16)
            # per (t, head): 128 cols = [v_h (64) | ones (64)] so one matmul
            # yields av rows 0:64 and the replicated softmax denominator
            # rows 64:128 in a single PSUM bank
            v2_sb = cpool.tile([128, TT, HPG, 128], F16)
            avT_sb = cpool.tile([128, 2, S], F16)
            wsrc_sb = cpool.tile([128, 512], F16)

            # ---- memsets + exp table load first: all off the DMA-issue
            # critical path, so the PE warm-up and the ACT table load run
            # during the initial DMA wait ----
            # gpsimd: its queue boots ~2us before DVE's, so the PE warm-up
            # (gated on wsrc) starts that much earlier
            nc.gpsimd.memset(wsrc_sb[:], 0.0)
            nc.gpsimd.memset(warm_sb[:], 0.0)
            # dense whole-tile memset (strided 1x memset costs 3.5us; the
            # v halves are overwritten by the v-projection drains)
            nc.vector.memset(
                v2_sb.rearrange("p t g d -> p (t g d)"), 1.0)
            # pull the exp table load off the critical path
            nc.scalar.activation(warm_sb[:], warm_sb[:], AF.Exp)

            # ---- DMAs in need-order: kT c0 inputs first (k-split), then
            # qT c0 inputs, then the rest. Each dma_start costs ~0.65us of
            # serial SP-sequencer issue time ----
            # critical transfers split into pieces spread across the
            # scalar/gpsimd/SP issue queues so they run in parallel; the
            # scalar+gpsimd issues also start ~1.5us before SP finishes
            # booting its own issue stream
            nc.scalar.dma_start(wk_sb[:, 0:3, :], wk[:, 0:3, :])
            nc.gpsimd.dma_start(ctxT_sb[:, 0:3, 0:512], ctxc[0, :, 0:3, :])
            nc.sync.dma_start(wk_sb[:, 3:6, :], wk[:, 3:6, :])
            nc.sync.dma_start(ctxT_sb[:, 3:6, 0:512], ctxc[0, :, 3:6, :])
            nc.scalar.dma_start(wq_sb[:, 0:4, :], wq[:, 0:4, :])
            nc.gpsimd.dma_start(xT_sb[:, 0:2, 0:512], xTc[0, :, 0:2, :])
            nc.sync.dma_start(xT_sb[:, 2:4, 0:512], xTc[0, :, 2:4, :])
            nc.sync.dma_start(bqk_sb[:], bqk[:])
            nc.scalar.dma_start(xT_sb[:, 4:6, 0:512], xTc[0, :, 4:6, :])
            nc.gpsimd.dma_start(xT_sb[:, 6:8, 0:512], xTc[0, :, 6:8, :])
            nc.sync.dma_start(wq_sb[:, 4:8, :], wq[:, 4:8, :])
            nc.sync.dma_start(wv_sb[:], wv[:])
            nc.sync.dma_start(bvr_sb[:], bvr[:])
            # per-chunk issues: JIT fillers then wait only on the chunk
            # they consume, not the whole remaining transfer
            for ch in range(1, SCK):
                csl = slice(ch * 512, (ch + 1) * 512)
                nc.sync.dma_start(ctxT_sb[:, :, csl], ctxc[ch, :, :, :])
            for ch in range(1, SCK):
                csl = slice(ch * 512, (ch + 1) * 512)
                nc.sync.dma_start(xT_sb[:, :, csl], xTc[ch, :, :, :])
            nc.sync.dma_start(wo_sb[:], wo[:])

            with (
                tc.tile_pool(name="psc", bufs=2, space="PSUM") as psc,
                tc.tile_pool(name="pproj", bufs=2, space="PSUM") as pproj,
                tc.tile_pool(name="pavz", bufs=1, space="PSUM") as pavz,
            ):
                uid = [0]

                def make_qk_units(dst, w_sb, bcol, src, nk, p, ch,
                                  drain_eng=None):
                    """One 512-col projection chunk as fine sub-units of
                    ~2 matmuls each; the last sub-unit drains PSUM->SBUF
                    with the bias add (DVE by default, ACT for prelude)."""
                    st = {}
                    csl = slice(ch * 512, (ch + 1) * 512)
                    wsl = slice(p * 128, (p + 1) * 128)
                    units = []
                    pairs = [(k, min(k + 2, nk)) for k in range(0, nk, 2)]

                    def mk(k0, k1, first, last):
                        def u():
                            if first:
                                uid[0] += 1
                                st["ps"] = pproj.tile([128, 512], F32,
                                                      tag="pj",
                                                      name=f"pj{uid[0]}")
                            for k in range(k0, k1):
                                nc.tensor.matmul(st["ps"][:], w_sb[:, k, wsl],
                                                 src[:, k, csl],
                                                 start=(k == 0), stop=False)
                            if last:
                                nc.tensor.matmul(st["ps"][:],
                                                 w_sb[:, nk - 1, wsl],
                                                 src[:, nk - 1, csl],
                                                 start=False, stop=True)
                                if drain_eng is nc.scalar:
                                    nc.scalar.activation(
                                        dst[:, p, csl], st["ps"][:],
                                        AF.Identity, bias=bcol)
                                else:
                                    nc.vector.tensor_tensor(
                                        dst[:, p, csl], st["ps"][:],
                                        bcol.to_broadcast([128, 512]),
                                        OP.add)
                        return u

                    # last matmul folded into the drain sub-unit
                    for i, (k0, k1) in enumerate(pairs):
                        last = i == len(pairs) - 1
                        units.append(mk(k0, k1 - (1 if last else 0),
                                        i == 0, last))
                    return units

                def make_v_units(t, ph, fused=False):
                    """v-projection for t-tile t, head-pair ph (2 heads,
                    128 cols of [v|v]): 6 K-tiles + DVE bias drain. fused
                    emits as one unit (prelude); else 3 sub-units."""
                    st = {}
                    hsl = slice(ph * 128, (ph + 1) * 128)
                    tsl = slice(t * 128, (t + 1) * 128)

                    def mk(k0, k1, first, last):
                        def u():
                            if first:
                                uid[0] += 1
                                st["ps"] = pproj.tile([128, 512], F32,
                                                      tag="pj",
                                                      name=f"pv{uid[0]}")
                            for k in range(k0, k1):
                                nc.tensor.matmul(
                                    st["ps"][:, 0:128], ctxT_sb[:, k, tsl],
                                    wv_sb[:, k, hsl],
                                    start=(k == 0), stop=(k == KT_C - 1))
                            if last:
                                nc.vector.tensor_tensor(
                                    v2_sb[:, t, 2 * ph:2 * ph + 2, 0:DH],
                                    st["ps"][:, 0:128].rearrange(
                                        "p (g d) -> p g d", d=DH),
                                    bvr_sb[:, hsl].rearrange(
                                        "p (g d) -> p g d", d=DH),
                                    OP.add)
                        return u

                    if fused:
                        return [mk(0, KT_C, True, True)]
                    return [mk(0, 2, True, False), mk(2, 4, False, False),
                            mk(4, KT_C, False, True)]

                def make_o_unit(sc, st_, n, cast_eng=None, dma_eng=None):
                    def u():
                        uid[0] += 1
                        row = (sc * 4 + st_) * 128
                        ps = pproj.tile([128, 512], F32, tag="pj",
                                        name=f"po{uid[0]}")
                        for l in range(2):
                            nc.tensor.matmul(
                                ps[:],
                                avT_sb[:, l, row:row + 128],
                                wo_sb[:, l, n * 512:(n + 1) * 512],
                                start=(l == 0), stop=(l == 1),
                            )
                        os_sb = ospool.tile([128, 512], F16, tag="os")
                        if cast_eng is nc.scalar:
                            nc.scalar.activation(os_sb[:], ps[:], AF.Copy)
                        else:
                            nc.vector.tensor_copy(os_sb[:], ps[:])
                        (dma_eng or nc.sync).dma_start(
                            out[row:row + 128, n * 512:(n + 1) * 512],
                            os_sb[:],
                        )
                    return u

                # ---- static filler schedule: sched[(w, t)] = [units] ----
                # deadline map (p-major windows (sc,p): w = 4*p + sc):
                #   v ph=0 tile t -> w0 slot t (race 2 ahead)
                #   v ph=1 tile t -> w4 slot t (spread over w1-w3)
                #   kT p chunk c  -> scores(w4p, 4c) issued at slot 4c-2
                #   qT p chunk c  -> start of window (c, p)
                #   o(sc)         -> after window (sc,1) = w4+sc
                sched = {}

                def put(w, t, u):
                    sched.setdefault((w, t), []).append(u)

                def put_seq(w, t0, units, stride=1):
                    for i, u in enumerate(units):
                        put(w, t0 + i * stride, u)

                # window 0: v ph0 tiles 2..15 racing 2 ahead; kT p0
                # c1/c2/c3 JIT; qT p0 c1 late
                for t in range(2, TT):
                    for u in make_v_units(t, 0, fused=True):
                        put(0, t - 2, u)
                # kT chunk c is read by scores(4c), issued at slot 4c-2;
                # the DVE drain needs ~a slot of margin before that
                for i, u in enumerate(make_qk_units(
                        kT_sb, wk_sb, bqk_sb[:, 2:3], ctxT_sb, KT_C, 0, 1)):
                    put(0, (0, 0, 1)[i], u)
                put_seq(0, 3, make_qk_units(kT_sb, wk_sb, bqk_sb[:, 2:3],
                                            ctxT_sb, KT_C, 0, 2))
                put_seq(0, 6, make_qk_units(kT_sb, wk_sb, bqk_sb[:, 2:3],
                                            ctxT_sb, KT_C, 0, 3))
                # drain must land ~2 slots before the w1 scores prefetch
                put_seq(0, 9, make_qk_units(qT_sb, wq_sb, bqk_sb[:, 0:1],
                                            xT_sb, KT_E, 0, 1))
                # window 1: kT p1 c0, qT p0 c2, v ph1 tiles 0-5
                put_seq(1, 1, make_qk_units(kT_sb, wk_sb, bqk_sb[:, 3:4],
                                            ctxT_sb, KT_C, 1, 0))
                put_seq(1, 4, make_qk_units(qT_sb, wq_sb, bqk_sb[:, 0:1],
                                            xT_sb, KT_E, 0, 2))
                for i, t in enumerate(range(0, 6)):
                    for u in make_v_units(t, 1, fused=True):
                        put(1, 8 + i, u)
                # window 2: qT p0 c3, kT p1 c1, v ph1 tiles 6-10
                put_seq(2, 1, make_qk_units(qT_sb, wq_sb, bqk_sb[:, 0:1],
                                            xT_sb, KT_E, 0, 3))
                put_seq(2, 5, make_qk_units(kT_sb, wk_sb, bqk_sb[:, 3:4],
                                            ctxT_sb, KT_C, 1, 1))
                for i, t in enumerate(range(6, 11)):
                    for u in make_v_units(t, 1, fused=True):
                        put(2, 9 + i, u)
                # window 3: qT p1 c0, kT p1 c2, v ph1 tiles 11-12
                put_seq(3, 1, make_qk_units(qT_sb, wq_sb, bqk_sb[:, 1:2],
                                            xT_sb, KT_E, 1, 0))
                put_seq(3, 5, make_qk_units(kT_sb, wk_sb, bqk_sb[:, 3:4],
                                            ctxT_sb, KT_C, 1, 2))
                for i, t in enumerate(range(11, 13)):
                    for u in make_v_units(t, 1, fused=True):
                        put(3, 9 + i, u)
                # window 4: kT p1 c3 JIT, v ph1 tiles 13-15 (deadline is
                # av(w4,t) so slots 3-7 are safe), qT p1 c1 at the end
                put_seq(4, 1, make_qk_units(kT_sb, wk_sb, bqk_sb[:, 3:4],
                                            ctxT_sb, KT_C, 1, 3))
                for i, t in enumerate(range(13, TT)):
                    for u in make_v_units(t, 1, fused=True):
                        put(4, 3 + 2 * i, u)
                put_seq(4, 8, make_qk_units(qT_sb, wq_sb, bqk_sb[:, 1:2],
                                            xT_sb, KT_E, 1, 1))
                # windows 5-7: out-proj of chunk (w-5) + remaining qT p1.
                # o-units start >= 2 slots into the window so the previous
                # window's normalize (DVE) has produced their avT rows; in
                # w7 they pack the tail end to keep the PE clock hot into
                # the final out-proj
                for sc in range(SCK - 1):
                    w = 5 + sc
                    base = 8 if w == 7 else 6
                    i = 0
                    for st_ in range(4):
                        for n in range(2):
                            put(w, base + i, make_o_unit(sc, st_, n))
                            i += 1
                put_seq(5, 2, make_qk_units(qT_sb, wq_sb, bqk_sb[:, 1:2],
                                            xT_sb, KT_E, 1, 2))
                put_seq(6, 2, make_qk_units(qT_sb, wq_sb, bqk_sb[:, 1:2],
                                            xT_sb, KT_E, 1, 3))

                # ---- PE p-state warm-up: dependency-free matmuls run
                # during the initial DMA wait so the continuous-busy ramp
                # to full clock completes before the real prelude. All
                # warm fills reuse ONE tile (a fresh pool tile per fill
                # would WAR a live projection PSUM and deadlock the
                # cross-engine drain ordering) ----
                wps = pproj.tile([128, 512], F32, tag="pj", name="warmps")
                for i in range(8):
                    nc.tensor.matmul(wps[:], wsrc_sb[:, 0:128],
                                     wsrc_sb[:, 0:512],
                                     start=(i == 0), stop=(i == 7))

                def warm_fill(n, tl=None):
                    for _ in range(n):
                        nc.tensor.matmul((tl if tl is not None else wps)[:],
                                         wsrc_sb[:, 0:128],
                                         wsrc_sb[:, 0:512],
                                         start=True, stop=True)

                # ---- prelude: minimum inputs for window 0; chunk-0
                # drains on the still-idle ACT engine; warm fills keep
                # the PE clock ramping through the DMA waits ----
                kt0 = make_qk_units(kT_sb, wk_sb, bqk_sb[:, 2:3],
                                    ctxT_sb, KT_C, 0, 0,
                                    drain_eng=nc.scalar)
                qt0 = make_qk_units(qT_sb, wq_sb, bqk_sb[:, 0:1],
                                    xT_sb, KT_E, 0, 0,
                                    drain_eng=nc.scalar)
                kt0[0]()
                warm_fill(2)
                kt0[1]()
                warm_fill(2)
                kt0[2]()
                # NOTE: anything emitted between here and qt0 delays the
                # chunk-0 drains (the scheduler coalesces the drain's
                # PE-counter wait past the later emissions) and costs
                # ~3-4us on the first EXP — keep this block minimal
                warm_fill(2)
                qt0[0]()
                qt0[1]()
                warm_fill(2)
                qt0[2]()
                qt0[3]()

                # ---- attention windows ----
                windows = [(sc, p) for p in range(2) for sc in range(SCK)]
                NW = len(windows)
                scps, exs = {}, {}

                def scores(w, t):
                    sc, p = windows[w]
                    scp = psc.tile([128, 1024], F32, tag="sc",
                                   name=f"sc{w}_{t}")
                    for h in range(2):
                        hb = h * DH
                        nc.tensor.matmul(
                            scp[:, h * 512:(h + 1) * 512],
                            kT_sb[hb:hb + DH, p, t * 128:(t + 1) * 128],
                            qT_sb[hb:hb + DH, p, sc * 512:(sc + 1) * 512],
                            start=True, stop=True,
                        )
                    scps[(w, t)] = scp

                def expf(w, t):
                    ex = expool.tile([128, 1024], F16, tag="ex",
                                     name=f"ex{w}_{t}")
                    nc.scalar.activation(ex[:], scps.pop((w, t))[:], AF.Exp,
                                         scale=0.125)
                    exs[(w, t)] = ex

                scores(0, 0)
                expf(0, 0)
                scores(0, 1)
                expf(0, 1)
                # v tiles 0/1 only gate av(w0,0/1), not the first EXP:
                # they run inside EXP(0)'s shadow instead of delaying it
                for u in make_v_units(0, 0, fused=True):
                    u()
                for u in make_v_units(1, 0, fused=True):
                    u()
                for w, (sc, p) in enumerate(windows):
                    last_w = w == NW - 1
                    ssl = slice(sc * 512, (sc + 1) * 512)
                    avz = pavz.tile([128, 1024], F32, tag="avz",
                                    name=f"avz{w}")

                    def av(t):
                        ex = exs.pop((w, t))
                        for h in range(2):
                            nc.tensor.matmul(
                                avz[:, h * 512:(h + 1) * 512],
                                v2_sb[:, t, p * 2 + h, :],
                                ex[:, h * 512:(h + 1) * 512],
                                start=(t == 0), stop=(t == TT - 1),
                            )

                    # scores/expf BEFORE av: both gate on end(EXP(t)) via
                    # the psc/ex RAW+WAR anyway, but scores-first keeps the
                    # ACT feed independent of av's avz-WAR on the previous
                    # window's freeing copy (kills the boundary bubble)
                    for t in range(TT):
                        for u in sched.pop((w, t), ()):
                            u()
                        if t + 2 < TT:
                            scores(w, t + 2)
                            expf(w, t + 2)
                        elif not last_w:
                            scores(w + 1, t - 14)
                            expf(w + 1, t - 14)
                        av(t)

                    if not last_w:
                        # normalize via base-0 staged reciprocal. The rz
                        # copy (2 halves so the WAR releases per-half)
                        # frees the avz PSUM pair for the next window's
                        # first av matmul; everything after reads SBUF.
                        rz = rzpool.tile([128, 1024], F32, tag="rz",
                                         name=f"rz{w}")
                        rzd = rzpool.tile([64, 2048], F32, tag="rzd",
                                          name=f"rzd{w}")
                        nc.vector.tensor_copy(rz[:, 0:512], avz[:, 0:512])
                        nc.vector.tensor_copy(rz[:, 512:1024],
                                              avz[:, 512:1024])
                        nc.vector.tensor_copy(rzd[0:DH, 0:1024],
                                              rz[DH:128, :])
                        for h in range(2):
                            hs = slice(h * 512, (h + 1) * 512)
                            nc.vector.reciprocal_approx_fast(
                                rzd[0:DH, 1024 + h * 512:1536 + h * 512],
                                rzd[0:DH, hs])
                        for h in range(2):
                            hb = h * DH
                            nc.vector.tensor_tensor(
                                avT_sb[hb:hb + DH, p, ssl],
                                rz[0:DH, h * 512:(h + 1) * 512],
                                rzd[0:DH, 1024 + h * 512:1536 + h * 512],
                                OP.mult,
                            )
                    else:
                        # final window: normalize in q-halves straight
                        # from avz PSUM (no freeing copy needed) and chase
                        # each half with its out-proj units so the tail
                        # overlaps. The denominator copy runs on the idle
                        # ACT engine in parallel with the DVE recip of the
                        # previous half; out-proj casts alternate ACT/DVE
                        # into a merged [128,1024] staging row so each st_
                        # needs one full-row (2KB-line) DMA, spread across
                        # the scalar/gpsimd/sync queues.
                        # keep the PE clocked through the normalize wait
                        wps2 = pproj.tile([128, 512], F32, tag="pj",
                                          name="warmps2")
                        for i in range(12):
                            nc.tensor.matmul(wps2[:], wsrc_sb[:, 0:128],
                                             wsrc_sb[:, 0:512],
                                             start=(i == 0), stop=(i == 11))
                        rzd = rzpool.tile([64, 2, 1024], F32, tag="rzd",
                                          name="rzdF")
                        osF = cpool.tile([128, 4, 1024], F16)
                        dma_engs = [nc.scalar, nc.sync, nc.scalar,
                                    nc.sync]
                        # full-width normalize: denominator copy on the
                        # idle ACT engine, then recip + two mults on DVE.
                        # (reciprocal_approx_fast straight from PSUM
                        # breaks on HW — the BITWISE_NOT seed needs an
                        # SBUF source; SIM does not model it)
                        nc.scalar.activation(rzd[0:DH, 0, :],
                                             avz[DH:128, :], AF.Copy)
                        nc.vector.reciprocal_approx_fast(
                            rzd[0:DH, 1, :], rzd[0:DH, 0, :])
                        for h in range(2):
                            hb = h * DH
                            nc.vector.tensor_tensor(
                                avT_sb[hb:hb + DH, p, ssl],
                                avz[0:DH, h * 512:(h + 1) * 512],
                                rzd[0:DH, 1, h * 512:(h + 1) * 512],
                                OP.mult,
                            )
                        # prestart st0's l=0 matmuls during the normalize
                        # (they read the long-finished avT p0 rows); the
                        # l=1 accumulation lands as soon as the mults do
                        row0 = sc * 4 * 128
                        pre = []
                        for n in range(2):
                            uid[0] += 1
                            ps = pproj.tile([128, 512], F32, tag="pj",
                                            name=f"poF{uid[0]}")
                            nc.tensor.matmul(
                                ps[:], avT_sb[:, 0, row0:row0 + 128],
                                wo_sb[:, 0, n * 512:(n + 1) * 512],
                                start=True, stop=False)
                            pre.append(ps)
                        for st_ in range(4):
                            row = (sc * 4 + st_) * 128
                            for n in range(2):
                                if st_ == 0:
                                    ps = pre[n]
                                else:
                                    uid[0] += 1
                                    ps = pproj.tile([128, 512], F32,
                                                    tag="pj",
                                                    name=f"poF{uid[0]}")
                                    nc.tensor.matmul(
                                        ps[:],
                                        avT_sb[:, 0, row:row + 128],
                                        wo_sb[:, 0,
                                              n * 512:(n + 1) * 512],
                                        start=True, stop=False)
                                nc.tensor.matmul(
                                    ps[:],
                                    avT_sb[:, 1, row:row + 128],
                                    wo_sb[:, 1, n * 512:(n + 1) * 512],
                                    start=False, stop=True)
                                dst = osF[:, st_, n * 512:(n + 1) * 512]
                                if n == 0:
                                    nc.scalar.activation(dst, ps[:],
                                                         AF.Copy)
                                else:
                                    nc.vector.tensor_copy(dst, ps[:])
                            dma_engs[st_].dma_start(
                                out[row:row + 128, :], osF[:, st_, :])

                assert not sched, f"unemitted fillers: {list(sched)}"

    nc.compile()
    return nc


def get_nc():
    if "nc" not in _NC_CACHE:
        _NC_CACHE["nc"] = _build_nc()
    return _NC_CACHE["nc"]


def make_in_maps(x, context, Wq, bq, Wk, bk, Wv, bv, Wo, bo):
    x = np.asarray(x, dtype=np.float32)
    context = np.asarray(context, dtype=np.float32)
    Wq = np.asarray(Wq, dtype=np.float32)
    Wk = np.asarray(Wk, dtype=np.float32)
    Wv = np.asarray(Wv, dtype=np.float32)
    Wo = np.asarray(Wo, dtype=np.float32)
    bq = np.asarray(bq, dtype=np.float32)
    bk = np.asarray(bk, dtype=np.float32)
    bv = np.asarray(bv, dtype=np.float32)

    # host-permuted chunk-major layouts: [SCK, 128, KT, 512] so every DMA
    # chunk is contiguous per partition (multi-KB lines)
    xTc = [np.ascontiguousarray(
        x[b].T.astype(np.float16).reshape(KT_E, 128, SCK, 512)
        .transpose(2, 1, 0, 3)) for b in range(B)]
    ctxc = [np.ascontiguousarray(
        context[b].T.astype(np.float16).reshape(KT_C, 128, SCK, 512)
        .transpose(2, 1, 0, 3)) for b in range(B)]
    in_maps = []
    for c in range(N_CORES):
        b, g = c // GROUPS, c % GROUPS
        sl = slice(g * DSL, (g + 1) * DSL)
        bqk = np.concatenate(
            [bq[sl].reshape(2, 128).T, bk[sl].reshape(2, 128).T], axis=1)
        in_maps.append({
            "xTc": xTc[b],
            "ctxc": ctxc[b],
            "wq": np.ascontiguousarray(
                Wq[:, sl].astype(np.float16).reshape(KT_E, 128, DSL)
                .transpose(1, 0, 2)),
            "wk": np.ascontiguousarray(
                Wk[:, sl].astype(np.float16).reshape(KT_C, 128, DSL)
                .transpose(1, 0, 2)),
            "wv": np.ascontiguousarray(
                Wv[:, sl].astype(np.float16).reshape(KT_C, 128, DSL)
                .transpose(1, 0, 2)),
            "wo": np.ascontiguousarray(
                Wo[sl, :].astype(np.float16).reshape(2, 128, E)
                .transpose(1, 0, 2)),
            "bqk": np.ascontiguousarray(bqk),
            "bvr": np.ascontiguousarray(
                np.tile(bv[sl].reshape(1, DSL), (128, 1))),
        })
    return in_maps


def run_sharded(inputs, trace=False):
    nc = get_nc()
    in_maps = make_in_maps(**inputs)
    res = bass_utils.run_bass_kernel_spmd(
        nc, in_maps, core_ids=list(range(N_CORES)), trace=trace,
    )
    bo = np.asarray(inputs["bo"], dtype=np.float32)
    full = np.empty((B, S, E), dtype=np.float32)
    for b in range(B):
        acc = res.results[b * GROUPS]["out"].astype(np.float32)
        for g in range(1, GROUPS):
            acc = acc + res.results[b * GROUPS + g]["out"].astype(np.float32)
        full[b] = acc + bo[None, :]
    return full, res.exec_time_ns


def kernel(**inputs) -> np.ndarray:
    return run_sharded(inputs)[0]


# revision 45
# speedup vs baseline: 1.0058x; 1.0058x over previous
"""Trainium2 Bass kernel for nn_CrossAttention (B=2, S=2048, E=1024, H=16, ctx=768).

Sharding: 4-way tensor-parallel over heads x 2-way data-parallel over batch.
Core c handles batch c//4 and heads 4*(c%4) .. 4*(c%4)+3.

Software-pipelined single-pass design, v2 (fine-grained schedule):
  PE is the near-saturated engine (~160us of matmul work vs ~142us of ACT
  exp work), so the schedule aims for: fast start (memsets before DMA
  issues, host-prepped contiguous DMA layouts split across the
  scalar/gpsimd/SP issue queues, chunk-0 PSUM drains on the idle ACT),
  per-t filler units of <=2 matmuls spread by deadline (v-projection
  split per head-pair so window 0 only carries its own heads' v; drains
  land >=2 slots before their consumer; o-units >=2 slots after the
  normalize that feeds them), next-window scores prefetched before av so
  the ACT stream crosses window boundaries without a bubble, and a
  compressed tail (full-width normalize with the denominator copy on the
  idle ACT engine, out-proj casts alternating ACT/DVE into merged
  full-row staging, output DMAs on the scalar+SP queues).

Per-core dataflow (fp16 operands, fp32 PSUM):
  qT/kT = W-stationary projections producing [dh, S] layouts (bias added
          during the PSUM->SBUF drain; DVE in-window, ACT for chunk 0)
  v2    = ctxT-tile-stationary projection into [S-tile, head, v|ones],
          one unit per (t, head-pair)
  scT   = kT-tile x qT (K=64); head pair emitted on PE row groups 0/64
  exp   = ScalarE, fused 1/sqrt(dh) scale, PSUM -> SBUF fp16, one
          [128,1024] tile per (t, head-pair); 1-deep scores lookahead
  av/Z  = [v_h | ones] stationary: rows 0:64 unnormalized out.T, rows
          64:128 softmax denominator, pair packed in one [128,1024] PSUM
  norm  = DVE: recip of denominator rows, multiply into avT fp16
  out   = avT-stationary x Wo, fp16 partial [S, E] per core

Host side: pre-transpose x/context, slice weights per head group, fp16
cast; sum the 4 per-batch fp16 partials + bo in fp32 on host.
"""
import numpy as np

import concourse.bass as bass
import concourse.mybir as mybir
import concourse.tile as tile
from concourse import bacc, bass_utils

F16 = mybir.dt.float16
F32 = mybir.dt.float32
AF = mybir.ActivationFunctionType
OP = mybir.AluOpType

B, S, E, C, H, DH = 2, 2048, 1024, 768, 16, 64
N_CORES = 8
GROUPS = 4            # head groups (tensor parallel)
HPG = H // GROUPS     # heads per group = 4
DSL = HPG * DH        # feature slice per core = 256
KT_E = E // 128       # 8 k-tiles for x projections
KT_C = C // 128       # 6 k-tiles for context projections
SCK = S // 512        # 4 s-chunks
TT = S // 128         # 16 t-tiles

_NC_CACHE = {}


def _build_nc():
    nc = bacc.Bacc("TRN2", target_bir_lowering=False, debug=False,
                   num_devices=N_CORES)

    # all inputs host-permuted into their exact SBUF layouts so every DMA
    # moves multi-KB contiguous lines (strided small-line DMAs measured
    # ~210 GB/s vs ~330+ contiguous)
    xTc = nc.dram_tensor("xTc", [SCK, 128, KT_E, 512], F16,
                         kind="ExternalInput").ap()
    ctxc = nc.dram_tensor("ctxc", [SCK, 128, KT_C, 512], F16,
                          kind="ExternalInput").ap()
    wq = nc.dram_tensor("wq", [128, KT_E, DSL], F16,
                        kind="ExternalInput").ap()
    wk = nc.dram_tensor("wk", [128, KT_C, DSL], F16,
                        kind="ExternalInput").ap()
    wv = nc.dram_tensor("wv", [128, KT_C, DSL], F16,
                        kind="ExternalInput").ap()
    wo = nc.dram_tensor("wo", [128, 2, E], F16, kind="ExternalInput").ap()
    bqk = nc.dram_tensor("bqk", [128, 4], F32, kind="ExternalInput").ap()
    bvr = nc.dram_tensor("bvr", [128, DSL], F32, kind="ExternalInput").ap()
    out = nc.dram_tensor("out", [S, E], F16, kind="ExternalOutput").ap()

    with tile.TileContext(nc) as tc:
        with (
            tc.tile_pool(name="const", bufs=1) as cpool,
            tc.tile_pool(name="ex", bufs=4) as expool,
            tc.tile_pool(name="os", bufs=4) as ospool,
            tc.tile_pool(name="rz", bufs=1) as rzpool,
        ):
            wq_sb = cpool.tile([128, KT_E, DSL], F16)
            wk_sb = cpool.tile([128, KT_C, DSL], F16)
            wv_sb = cpool.tile([128, KT_C, DSL], F16)
            wo_sb = cpool.tile([128, 2, E], F16)
            bqk_sb = cpool.tile([128, 4], F32)
            bvr_sb = cpool.tile([128, DSL], F32)
            warm_sb = cpool.tile([1, 8], F32)
            ctxT_sb = cpool.tile([128, KT_C, S], F16)
            xT_sb = cpool.tile([128, KT_E, S], F16)

            qT_sb = cpool.tile([128, 2, S], F16)
            kT_sb = cpool.tile([128, 2, S], F16)
            # per (t, head): 128 cols = [v_h (64) | ones (64)] so one matmul
            # yields av rows 0:64 and the replicated softmax denominator
            # rows 64:128 in a single PSUM bank
            v2_sb = cpool.tile([128, TT, HPG, 128], F16)
            avT_sb = cpool.tile([128, 2, S], F16)
            wsrc_sb = cpool.tile([128, 512], F16)

            # ---- memsets + exp table load first: all off the DMA-issue
            # critical path, so the PE warm-up and the ACT table load run
            # during the initial DMA wait ----
            # gpsimd: its queue boots ~2us before DVE's, so the PE warm-up
            # (gated on wsrc) starts that much earlier
            nc.gpsimd.memset(wsrc_sb[:], 0.0)
            nc.gpsimd.memset(warm_sb[:], 0.0)
            # dense whole-tile memset (strided 1x memset costs 3.5us; the
            # v halves are overwritten by the v-projection drains)
            nc.vector.memset(
                v2_sb.rearrange("p t g d -> p (t g d)"), 1.0)
            # pull the exp table load off the critical path
            nc.scalar.activation(warm_sb[:], warm_sb[:], AF.Exp)

            # ---- DMAs in need-order: kT c0 inputs first (k-split), then
            # qT c0 inputs, then the rest. Each dma_start costs ~0.65us of
            # serial SP-sequencer issue time ----
            # critical transfers split into pieces spread across the
            # scalar/gpsimd/SP issue queues so they run in parallel; the
            # scalar+gpsimd issues also start ~1.5us before SP finishes
            # booting its own issue stream
            nc.scalar.dma_start(wk_sb[:, 0:3, :], wk[:, 0:3, :])
            nc.gpsimd.dma_start(ctxT_sb[:, 0:3, 0:512], ctxc[0, :, 0:3, :])
            nc.sync.dma_start(wk_sb[:, 3:6, :], wk[:, 3:6, :])
            nc.sync.dma_start(ctxT_sb[:, 3:6, 0:512], ctxc[0, :, 3:6, :])
            nc.scalar.dma_start(wq_sb[:, 0:4, :], wq[:, 0:4, :])
            nc.gpsimd.dma_start(xT_sb[:, 0:2, 0:512], xTc[0, :, 0:2, :])
            nc.sync.dma_start(xT_sb[:, 2:4, 0:512], xTc[0, :, 2:4, :])
            nc.sync.dma_start(bqk_sb[:], bqk[:])
            nc.scalar.dma_start(xT_sb[:, 4:6, 0:512], xTc[0, :, 4:6, :])
            nc.gpsimd.dma_start(xT_sb[:, 6:8, 0:512], xTc[0, :, 6:8, :])
            nc.sync.dma_start(wq_sb[:, 4:8, :], wq[:, 4:8, :])
            nc.sync.dma_start(wv_sb[:], wv[:])
            nc.sync.dma_start(bvr_sb[:], bvr[:])
            # per-chunk issues: JIT fillers then wait only on the chunk
            # they consume, not the whole remaining transfer
            for ch in range(1, SCK):
                csl = slice(ch * 512, (ch + 1) * 512)
                nc.sync.dma_start(ctxT_sb[:, :, csl], ctxc[ch, :, :, :])
            for ch in range(1, SCK):
                csl = slice(ch * 512, (ch + 1) * 512)
                nc.sync.dma_start(xT_sb[:, :, csl], xTc[ch, :, :, :])
            nc.sync.dma_start(wo_sb[:], wo[:])

            with (
                tc.tile_pool(name="psc", bufs=2, space="PSUM") as psc,
                tc.tile_pool(name="pproj", bufs=2, space="PSUM") as pproj,
                tc.tile_pool(name="pavz", bufs=1, space="PSUM") as pavz,
            ):
                uid = [0]

                def make_qk_units(dst, w_sb, bcol, src, nk, p, ch,
                                  drain_eng=None):
                    """One 512-col projection chunk as fine sub-units of
                    ~2 matmuls each; the last sub-unit drains PSUM->SBUF
                    with the bias add (DVE by default, ACT for prelude)."""
                    st = {}
                    csl = slice(ch * 512, (ch + 1) * 512)
                    wsl = slice(p * 128, (p + 1) * 128)
                    units = []
                    pairs = [(k, min(k + 2, nk)) for k in range(0, nk, 2)]

                    def mk(k0, k1, first, last):
                        def u():
                            if first:
                                uid[0] += 1
                                st["ps"] = pproj.tile([128, 512], F32,
                                                      tag="pj",
                                                      name=f"pj{uid[0]}")
                            for k in range(k0, k1):
                                nc.tensor.matmul(st["ps"][:], w_sb[:, k, wsl],
                                                 src[:, k, csl],
                                                 start=(k == 0), stop=False)
                            if last:
                                nc.tensor.matmul(st["ps"][:],
                                                 w_sb[:, nk - 1, wsl],
                                                 src[:, nk - 1, csl],
                                                 start=False, stop=True)
                                if drain_eng is nc.scalar:
                                    nc.scalar.activation(
                                        dst[:, p, csl], st["ps"][:],
                                        AF.Identity, bias=bcol)
                                else:
                                    nc.vector.tensor_tensor(
                                        dst[:, p, csl], st["ps"][:],
                                        bcol.to_broadcast([128, 512]),
                                        OP.add)
                        return u

                    # last matmul folded into the drain sub-unit
                    for i, (k0, k1) in enumerate(pairs):
                        last = i == len(pairs) - 1
                        units.append(mk(k0, k1 - (1 if last else 0),
                                        i == 0, last))
                    return units

                def make_v_units(t, ph, fused=False):
                    """v-projection for t-tile t, head-pair ph (2 heads,
                    128 cols of [v|v]): 6 K-tiles + DVE bias drain. fused
                    emits as one unit (prelude); else 3 sub-units."""
                    st = {}
                    hsl = slice(ph * 128, (ph + 1) * 128)
                    tsl = slice(t * 128, (t + 1) * 128)

                    def mk(k0, k1, first, last):
                        def u():
                            if first:
                                uid[0] += 1
                                st["ps"] = pproj.tile([128, 512], F32,
                                                      tag="pj",
                                                      name=f"pv{uid[0]}")
                            for k in range(k0, k1):
                                nc.tensor.matmul(
                                    st["ps"][:, 0:128], ctxT_sb[:, k, tsl],
                                    wv_sb[:, k, hsl],
                                    start=(k == 0), stop=(k == KT_C - 1))
                            if last:
                                nc.vector.tensor_tensor(
                                    v2_sb[:, t, 2 * ph:2 * ph + 2, 0:DH],
                                    st["ps"][:, 0:128].rearrange(
                                        "p (g d) -> p g d", d=DH),
                                    bvr_sb[:, hsl].rearrange(
                                        "p (g d) -> p g d", d=DH),
                                    OP.add)
                        return u

                    if fused:
                        return [mk(0, KT_C, True, True)]
                    return [mk(0, 2, True, False), mk(2, 4, False, False),
                            mk(4, KT_C, False, True)]

                def make_o_unit(sc, st_, n, cast_eng=None, dma_eng=None):
                    def u():
                        uid[0] += 1
                        row = (sc * 4 + st_) * 128
                        ps = pproj.tile([128, 512], F32, tag="pj",
                                        name=f"po{uid[0]}")
                        for l in range(2):
                            nc.tensor.matmul(
                                ps[:],
                                avT_sb[:, l, row:row + 128],
                                wo_sb[:, l, n * 512:(n + 1) * 512],
                                start=(l == 0), stop=(l == 1),
                            )
                        os_sb = ospool.tile([128, 512], F16, tag="os")
                        if cast_eng is nc.scalar:
                            nc.scalar.activation(os_sb[:], ps[:], AF.Copy)
                        else:
                            nc.vector.tensor_copy(os_sb[:], ps[:])
                        (dma_eng or nc.sync).dma_start(
                            out[row:row + 128, n * 512:(n + 1) * 512],
                            os_sb[:],
                        )
                    return u

                # ---- static filler schedule: sched[(w, t)] = [units] ----
                # deadline map (p-major windows (sc,p): w = 4*p + sc):
                #   v ph=0 tile t -> w0 slot t (race 2 ahead)
                #   v ph=1 tile t -> w4 slot t (spread over w1-w3)
                #   kT p chunk c  -> scores(w4p, 4c) issued at slot 4c-2
                #   qT p chunk c  -> start of window (c, p)
                #   o(sc)         -> after window (sc,1) = w4+sc
                sched = {}

                def put(w, t, u):
                    sched.setdefault((w, t), []).append(u)

                def put_seq(w, t0, units, stride=1):
                    for i, u in enumerate(units):
                        put(w, t0 + i * stride, u)

                # window 0: v ph0 tiles 2..15 racing 2 ahead; kT p0
                # c1/c2/c3 JIT; qT p0 c1 late
                for t in range(2, TT):
                    for u in make_v_units(t, 0, fused=True):
                        put(0, t - 2, u)
                # kT chunk c is read by scores(4c), issued at slot 4c-2;
                # the DVE drain needs ~a slot of margin before that
                for i, u in enumerate(make_qk_units(
                        kT_sb, wk_sb, bqk_sb[:, 2:3], ctxT_sb, KT_C, 0, 1)):
                    put(0, (0, 0, 1)[i], u)
                put_seq(0, 3, make_qk_units(kT_sb, wk_sb, bqk_sb[:, 2:3],
                                            ctxT_sb, KT_C, 0, 2))
                put_seq(0, 6, make_qk_units(kT_sb, wk_sb, bqk_sb[:, 2:3],
                                            ctxT_sb, KT_C, 0, 3))
                # drain must land ~2 slots before the w1 scores prefetch
                put_seq(0, 9, make_qk_units(qT_sb, wq_sb, bqk_sb[:, 0:1],
                                            xT_sb, KT_E, 0, 1))
                # window 1: kT p1 c0, qT p0 c2, v ph1 tiles 0-5
                put_seq(1, 1, make_qk_units(kT_sb, wk_sb, bqk_sb[:, 3:4],
                                            ctxT_sb, KT_C, 1, 0))
                put_seq(1, 4, make_qk_units(qT_sb, wq_sb, bqk_sb[:, 0:1],
                                            xT_sb, KT_E, 0, 2))
                for i, t in enumerate(range(0, 6)):
                    for u in make_v_units(t, 1, fused=True):
                        put(1, 8 + i, u)
                # window 2: qT p0 c3, kT p1 c1, v ph1 tiles 6-10
                put_seq(2, 1, make_qk_units(qT_sb, wq_sb, bqk_sb[:, 0:1],
                                            xT_sb, KT_E, 0, 3))
                put_seq(2, 5, make_qk_units(kT_sb, wk_sb, bqk_sb[:, 3:4],
                                            ctxT_sb, KT_C, 1, 1))
                for i, t in enumerate(range(6, 11)):
                    for u in make_v_units(t, 1, fused=True):
                        put(2, 9 + i, u)
                # window 3: qT p1 c0, kT p1 c2, v ph1 tiles 11-12
                put_seq(3, 1, make_qk_units(qT_sb, wq_sb, bqk_sb[:, 1:2],
                                            xT_sb, KT_E, 1, 0))
                put_seq(3, 5, make_qk_units(kT_sb, wk_sb, bqk_sb[:, 3:4],
                                            ctxT_sb, KT_C, 1, 2))
                for i, t in enumerate(range(11, 13)):
                    for u in make_v_units(t, 1, fused=True):
                        put(3, 9 + i, u)
                # window 4: kT p1 c3 JIT, v ph1 tiles 13-15 (deadline is
                # av(w4,t) so slots 3-7 are safe), qT p1 c1 at the end
                put_seq(4, 1, make_qk_units(kT_sb, wk_sb, bqk_sb[:, 3:4],
                                            ctxT_sb, KT_C, 1, 3))
                for i, t in enumerate(range(13, TT)):
                    for u in make_v_units(t, 1, fused=True):
                        put(4, 3 + 2 * i, u)
                put_seq(4, 8, make_qk_units(qT_sb, wq_sb, bqk_sb[:, 1:2],
                                            xT_sb, KT_E, 1, 1))
                # windows 5-7: out-proj of chunk (w-5) + remaining qT p1.
                # o-units start >= 2 slots into the window so the previous
                # window's normalize (DVE) has produced their avT rows; in
                # w7 they pack the tail end to keep the PE clock hot into
                # the final out-proj
                for sc in range(SCK - 1):
                    w = 5 + sc
                    base = 8 if w == 7 else 6
                    i = 0
                    for st_ in range(4):
                        for n in range(2):
                            put(w, base + i, make_o_unit(sc, st_, n))
                            i += 1
                put_seq(5, 2, make_qk_units(qT_sb, wq_sb, bqk_sb[:, 1:2],
                                            xT_sb, KT_E, 1, 2))
                put_seq(6, 2, make_qk_units(qT_sb, wq_sb, bqk_sb[:, 1:2],
                                            xT_sb, KT_E, 1, 3))

                # ---- PE p-state warm-up: dependency-free matmuls run
                # during the initial DMA wait so the continuous-busy ramp
                # to full clock completes before the real prelude. All
                # warm fills reuse ONE tile (a fresh pool tile per fill
                # would WAR a live projection PSUM and deadlock the
                # cross-engine drain ordering) ----
                wps = pproj.tile([128, 512], F32, tag="pj", name="warmps")
                for i in range(8):
                    nc.tensor.matmul(wps[:], wsrc_sb[:, 0:128],
                                     wsrc_sb[:, 0:512],
                                     start=(i == 0), stop=(i == 7))

                def warm_fill(n, tl=None):
                    for _ in range(n):
                        nc.tensor.matmul((tl if tl is not None else wps)[:],
                                         wsrc_sb[:, 0:128],
                                         wsrc_sb[:, 0:512],
                                         start=True, stop=True)

                # ---- prelude: minimum inputs for window 0; chunk-0
                # drains on the still-idle ACT engine; warm fills keep
                # the PE clock ramping through the DMA waits ----
                kt0 = make_qk_units(kT_sb, wk_sb, bqk_sb[:, 2:3],
                                    ctxT_sb, KT_C, 0, 0,
                                    drain_eng=nc.scalar)
                qt0 = make_qk_units(qT_sb, wq_sb, bqk_sb[:, 0:1],
                                    xT_sb, KT_E, 0, 0,
                                    drain_eng=nc.scalar)
                kt0[0]()
                warm_fill(2)
                kt0[1]()
                warm_fill(2)
                kt0[2]()
                # NOTE: anything emitted between here and qt0 delays the
                # chunk-0 drains (the scheduler coalesces the drain's
                # PE-counter wait past the later emissions) and costs
                # ~3-4us on the first EXP — keep this block minimal
                warm_fill(2)
                qt0[0]()
                qt0[1]()
                warm_fill(2)
                qt0[2]()
                qt0[3]()

                # ---- attention windows ----
                windows = [(sc, p) for p in range(2) for sc in range(SCK)]
                NW = len(windows)
                scps, exs = {}, {}

                def scores(w, t):
                    sc, p = windows[w]
                    scp = psc.tile([128, 1024], F32, tag="sc",
                                   name=f"sc{w}_{t}")
                    for h in range(2):
                        hb = h * DH
                        nc.tensor.matmul(
                            scp[:, h * 512:(h + 1) * 512],
                            kT_sb[hb:hb + DH, p, t * 128:(t + 1) * 128],
                            qT_sb[hb:hb + DH, p, sc * 512:(sc + 1) * 512],
                            start=True, stop=True,
                        )
                    scps[(w, t)] = scp

                def expf(w, t):
                    ex = expool.tile([128, 1024], F16, tag="ex",
                                     name=f"ex{w}_{t}")
                    nc.scalar.activation(ex[:], scps.pop((w, t))[:], AF.Exp,
                                         scale=0.125)
                    exs[(w, t)] = ex

                scores(0, 0)
                expf(0, 0)
                scores(0, 1)
                expf(0, 1)
                # v tiles 0/1 only gate av(w0,0/1), not the first EXP:
                # they run inside EXP(0)'s shadow instead of delaying it
                for u in make_v_units(0, 0, fused=True):
                    u()
                for u in make_v_units(1, 0, fused=True):
                    u()
                for w, (sc, p) in enumerate(windows):
                    last_w = w == NW - 1
                    ssl = slice(sc * 512, (sc + 1) * 512)
                    avz = pavz.tile([128, 1024], F32, tag="avz",
                                    name=f"avz{w}")

                    def av(t):
                        ex = exs.pop((w, t))
                        for h in range(2):
                            nc.tensor.matmul(
                                avz[:, h * 512:(h + 1) * 512],
                                v2_sb[:, t, p * 2 + h, :],
                                ex[:, h * 512:(h + 1) * 512],
                                start=(t == 0), stop=(t == TT - 1),
                            )

                    # scores/expf BEFORE av: both gate on end(EXP(t)) via
                    # the psc/ex RAW+WAR anyway, but scores-first keeps the
                    # ACT feed independent of av's avz-WAR on the previous
                    # window's freeing copy (kills the boundary bubble)
                    for t in range(TT):
                        for u in sched.pop((w, t), ()):
                            u()
                        if t + 2 < TT:
                            scores(w, t + 2)
                            expf(w, t + 2)
                        elif not last_w:
                            scores(w + 1, t - 14)
                            expf(w + 1, t - 14)
                        av(t)

                    if not last_w:
                        # normalize via base-0 staged reciprocal. The rz
                        # copy (2 halves so the WAR releases per-half)
                        # frees the avz PSUM pair for the next window's
                        # first av matmul; everything after reads SBUF.
                        rz = rzpool.tile([128, 1024], F32, tag="rz",
                                         name=f"rz{w}")
                        rzd = rzpool.tile([64, 2048], F32, tag="rzd",
                                          name=f"rzd{w}")
                        nc.vector.tensor_copy(rz[:, 0:512], avz[:, 0:512])
                        nc.vector.tensor_copy(rz[:, 512:1024],
                                              avz[:, 512:1024])
                        nc.vector.tensor_copy(rzd[0:DH, 0:1024],
                                              rz[DH:128, :])
                        for h in range(2):
                            hs = slice(h * 512, (h + 1) * 512)
                            nc.vector.reciprocal_approx_fast(
                                rzd[0:DH, 1024 + h * 512:1536 + h * 512],
                                rzd[0:DH, hs])
                        for h in range(2):
                            hb = h * DH
                            nc.vector.tensor_tensor(
                                avT_sb[hb:hb + DH, p, ssl],
                                rz[0:DH, h * 512:(h + 1) * 512],
                                rzd[0:DH, 1024 + h * 512:1536 + h * 512],
                                OP.mult,
                            )
                    else:
                        # final window: normalize in q-halves straight
                        # from avz PSUM (no freeing copy needed) and chase
                        # each half with its out-proj units so the tail
                        # overlaps. The denominator copy runs on the idle
                        # ACT engine in parallel with the DVE recip of the
                        # previous half; out-proj casts alternate ACT/DVE
                        # into a merged [128,1024] staging row so each st_
                        # needs one full-row (2KB-line) DMA, spread across
                        # the scalar/gpsimd/sync queues.
                        # keep the PE clocked through the normalize wait
                        wps2 = pproj.tile([128, 512], F32, tag="pj",
                                          name="warmps2")
                        for i in range(12):
                            nc.tensor.matmul(wps2[:], wsrc_sb[:, 0:128],
                                             wsrc_sb[:, 0:512],
                                             start=(i == 0), stop=(i == 11))
                        rzd = rzpool.tile([64, 2, 1024], F32, tag="rzd",
                                          name="rzdF")
                        osF = cpool.tile([128, 4, 1024], F16)
                        dma_engs = [nc.scalar, nc.sync, nc.scalar,
                                    nc.sync]
                        # full-width normalize: denominator copy on the
                        # idle ACT engine, then recip + two mults on DVE.
                        # (reciprocal_approx_fast straight from PSUM
                        # breaks on HW — the BITWISE_NOT seed needs an
                        # SBUF source; SIM does not model it)
                        nc.scalar.activation(rzd[0:DH, 0, :],
                                             avz[DH:128, :], AF.Copy)
                        nc.vector.reciprocal_approx_fast(
                            rzd[0:DH, 1, :], rzd[0:DH, 0, :])
                        # mults split by q-half: st0/st1 (first 256 q)
                        # start after the first two quarter-mults instead
                        # of waiting for the full-width pair
                        for qh in range(2):
                            for h in range(2):
                                hb = h * DH
                                qa = h * 512 + qh * 256
                                nc.vector.tensor_tensor(
                                    avT_sb[hb:hb + DH, p,
                                           sc * 512 + qh * 256:
                                           sc * 512 + (qh + 1) * 256],
                                    avz[0:DH, qa:qa + 256],
                                    rzd[0:DH, 1, qa:qa + 256],
                                    OP.mult,
                                )
                        # prestart st0's l=0 matmuls during the normalize
                        # (they read the long-finished avT p0 rows); the
                        # l=1 accumulation lands as soon as the mults do
                        row0 = sc * 4 * 128
                        pre = []
                        for n in range(2):
                            uid[0] += 1
                            ps = pproj.tile([128, 512], F32, tag="pj",
                                            name=f"poF{uid[0]}")
                            nc.tensor.matmul(
                                ps[:], avT_sb[:, 0, row0:row0 + 128],
                                wo_sb[:, 0, n * 512:(n + 1) * 512],
                                start=True, stop=False)
                            pre.append(ps)
                        for st_ in range(4):
                            row = (sc * 4 + st_) * 128
                            for n in range(2):
                                if st_ == 0:
                                    ps = pre[n]
                                else:
                                    uid[0] += 1
                                    ps = pproj.tile([128, 512], F32,
                                                    tag="pj",
                                                    name=f"poF{uid[0]}")
                                    nc.tensor.matmul(
                                        ps[:],
                                        avT_sb[:, 0, row:row + 128],
                                        wo_sb[:, 0,
                                              n * 512:(n + 1) * 512],
                                        start=True, stop=False)
                                nc.tensor.matmul(
                                    ps[:],
                                    avT_sb[:, 1, row:row + 128],
                                    wo_sb[:, 1, n * 512:(n + 1) * 512],
                                    start=False, stop=True)
                                dst = osF[:, st_, n * 512:(n + 1) * 512]
                                if n == 0:
                                    nc.scalar.activation(dst, ps[:],
                                                         AF.Copy)
                                else:
                                    nc.vector.tensor_copy(dst, ps[:])
                            dma_engs[st_].dma_start(
                                out[row:row + 128, :], osF[:, st_, :])

                assert not sched, f"unemitted fillers: {list(sched)}"

    nc.compile()
    return nc


def get_nc():
    if "nc" not in _NC_CACHE:
        _NC_CACHE["nc"] = _build_nc()
    return _NC_CACHE["nc"]


def make_in_maps(x, context, Wq, bq, Wk, bk, Wv, bv, Wo, bo):
    x = np.asarray(x, dtype=np.float32)
    context = np.asarray(context, dtype=np.float32)
    Wq = np.asarray(Wq, dtype=np.float32)
    Wk = np.asarray(Wk, dtype=np.float32)
    Wv = np.asarray(Wv, dtype=np.float32)
    Wo = np.asarray(Wo, dtype=np.float32)
    bq = np.asarray(bq, dtype=np.float32)
    bk = np.asarray(bk, dtype=np.float32)
    bv = np.asarray(bv, dtype=np.float32)

    # host-permuted chunk-major layouts: [SCK, 128, KT, 512] so every DMA
    # chunk is contiguous per partition (multi-KB lines)
    xTc = [np.ascontiguousarray(
        x[b].T.astype(np.float16).reshape(KT_E, 128, SCK, 512)
        .transpose(2, 1, 0, 3)) for b in range(B)]
    ctxc = [np.ascontiguousarray(
        context[b].T.astype(np.float16).reshape(KT_C, 128, SCK, 512)
        .transpose(2, 1, 0, 3)) for b in range(B)]
    in_maps = []
    for c in range(N_CORES):
        b, g = c // GROUPS, c % GROUPS
        sl = slice(g * DSL, (g + 1) * DSL)
        bqk = np.concatenate(
            [bq[sl].reshape(2, 128).T, bk[sl].reshape(2, 128).T], axis=1)
        in_maps.append({
            "xTc": xTc[b],
            "ctxc": ctxc[b],
            "wq": np.ascontiguousarray(
                Wq[:, sl].astype(np.float16).reshape(KT_E, 128, DSL)
                .transpose(1, 0, 2)),
            "wk": np.ascontiguousarray(
                Wk[:, sl].astype(np.float16).reshape(KT_C, 128, DSL)
                .transpose(1, 0, 2)),
            "wv": np.ascontiguousarray(
                Wv[:, sl].astype(np.float16).reshape(KT_C, 128, DSL)
                .transpose(1, 0, 2)),
            "wo": np.ascontiguousarray(
                Wo[sl, :].astype(np.float16).reshape(2, 128, E)
                .transpose(1, 0, 2)),
            "bqk": np.ascontiguousarray(bqk),
            "bvr": np.ascontiguousarray(
                np.tile(bv[sl].reshape(1, DSL), (128, 1))),
        })
    return in_maps


def run_sharded(inputs, trace=False):
    nc = get_nc()
    in_maps = make_in_maps(**inputs)
    res = bass_utils.run_bass_kernel_spmd(
        nc, in_maps, core_ids=list(range(N_CORES)), trace=trace,
    )
    bo = np.asarray(inputs["bo"], dtype=np.float32)
    full = np.empty((B, S, E), dtype=np.float32)
    for b in range(B):
        acc = res.results[b * GROUPS]["out"].astype(np.float32)
        for g in range(1, GROUPS):
            acc = acc + res.results[b * GROUPS + g]["out"].astype(np.float32)
        full[b] = acc + bo[None, :]
    return full, res.exec_time_ns


def kernel(**inputs) -> np.ndarray:
    return run_sharded(inputs)[0]
